# revision 25
# baseline (speedup 1.0000x reference)
"""GATv2 2-layer GNN on 8 Trainium2 NeuronCores (Bass/Tile, edge-parallel).

Sharding: edges sorted by dst node, dst-range sharded across 8 cores
(core k owns dst nodes [1250k, 1250(k+1))), so the per-dst segment
softmax and aggregation are fully core-local. Self-loop edge attrs
(mean of incoming) are precomputed host-side and folded into the edge
stream. Layer-1 node projections are computed replicated (xl1 for all
nodes; xr1 for the own slice). Between layers only the 32-wide xl2
projection is exchanged with a single AllGather (xr2 is dst-local).

Per-edge-chunk pipeline (128 edges):
  gpsimd: q = xl1[src] (+)DMA xr1[dst]          (fused add-gather, bf16)
  tensor: ep = ea @ We1 into PSUM (2x 1024-wide matmuls)
  vector: z = q + ep ; 8x logit-accum STT ; 8x xls = q*ex
  scalar: m = LeakyRelu(z) ; ex = Exp(logits)
  tensor: acc += s01^T @ xls ; den += s01^T @ ex
Aggregation identity: sum_e alpha*(xl+xr) = out[d] + xr[d] (softmax
sums to 1), so the group finalize computes h = relu(acc/den - xr[d])
and xl never needs to be gathered standalone.
"""
import sys
sys.path.insert(0, "/opt/trn_rl_repo")

import numpy as np
import ml_dtypes

import concourse.bass as bass
import concourse.bacc as bacc
import concourse.tile as tile
from concourse import mybir
from concourse.bass_utils import run_bass_kernel_spmd

BF16 = ml_dtypes.bfloat16

N, E, F = 10000, 80000, 128
H1, C1 = 8, 256
D1 = H1 * C1          # 2048
D2 = 32               # layer-2 out (1 head)
NEG = 0.2
M = 8                 # cores
NPC = N // M          # 1250 nodes per core
GN = 125              # dst nodes per group
G = NPC // GN         # 10 groups per core
P = 128

dt = mybir.dt


def _build_program(CH):
    """Build the SPMD Bass program. CH = chunks per group (self edges incl)."""
    NCH = G * CH          # chunks per core
    L = NCH * P           # edge-stream length per core
    nc = bacc.Bacc("TRN2", target_bir_lowering=False, debug=False, num_devices=M)

    ei = {}
    def EIN(name, shape, dtype):
        ei[name] = nc.dram_tensor(name, list(shape), dtype, kind="ExternalInput")
        return ei[name]

    xT    = EIN("xT",    (P, N),       dt.bfloat16)   # x transposed (replicated)
    xsT   = EIN("xsT",   (P, G * P),   dt.bfloat16)   # own-slice cols, group padded
    wl1   = EIN("wl1",   (F, D1),      dt.bfloat16)
    wr1   = EIN("wr1",   (F, D1),      dt.bfloat16)
    we1   = EIN("we1",   (F, D1),      dt.bfloat16)
    att1r = EIN("att1r", (P, 2 * D1),  dt.bfloat16)   # att1 rows, tiled twice
    wlr2  = EIN("wlr2",  (P, 16 * 2 * D2), dt.bfloat16)  # [p, k*64+c]: Wl2|Wr2
    we2   = EIN("we2",   (F, D2),      dt.bfloat16)
    att2r = EIN("att2r", (P, 2 * D2),  dt.bfloat16)   # att2 rows, tiled twice
    ident = EIN("ident", (P, P),       dt.bfloat16)   # identity (PE transpose)
    eaT   = EIN("eaT",   (F, L),       dt.bfloat16)   # edge_attr^T, sorted+padded
    s01T  = EIN("s01T",  (P, L),       dt.bfloat16)   # [p, ch*128+d]: edge p of
                                                      # chunk ch one-hot on dst d
    srci  = EIN("srci",  (P, NCH),     dt.int32)      # global src id per slot
    dstpi = EIN("dstpi", (P, NCH),     dt.int32)      # group-padded local dst id
    dstli = EIN("dstli", (P, NCH),     dt.int32)      # within-group local dst id

    out = nc.dram_tensor("out", [NPC, D2], dt.float32, kind="ExternalOutput")

    # DRAM scratch
    xl1_tab = nc.dram_tensor("xl1_tab", [N, D1], dt.bfloat16)
    xr1_sl  = nc.dram_tensor("xr1_sl", [G * P, D1], dt.bfloat16)
    xl2_own = nc.dram_tensor("xl2_own", [NPC, D2], dt.float32)
    xr2_tabs = [nc.dram_tensor(f"xr2_tab{g}", [P, D2], dt.float32)
                for g in range(G)]
    xl2_all = nc.dram_tensor("xl2_all", [N, D2], dt.float32, addr_space="Shared")
    dum0    = nc.dram_tensor("dum0", [1, 8], dt.bfloat16)
    dumAG   = nc.dram_tensor("dumAG", [M, 8], dt.bfloat16, addr_space="Shared")
    dumsink = nc.dram_tensor("dumsink", [1, 8], dt.bfloat16)

    AF = mybir.ActivationFunctionType
    ALU = mybir.AluOpType
    RG = [list(range(M))]

    with tile.TileContext(nc) as tc:
        with tc.tile_pool(name="consts", bufs=1) as cp:
            def CONST(name, src, shape, dtype):
                t = cp.tile(list(shape), dtype, name=name)
                nc.sync.dma_start(out=t[:], in_=src[:])
                return t

            xT_sb   = CONST("xT_sb", xT, (P, N), dt.bfloat16)
            xsT_sb  = CONST("xsT_sb", xsT, (P, G * P), dt.bfloat16)
            wl1_sb  = CONST("wl1_sb", wl1, (F, D1), dt.bfloat16)
            wr1_sb  = CONST("wr1_sb", wr1, (F, D1), dt.bfloat16)
            we1_sb  = CONST("we1_sb", we1, (F, D1), dt.bfloat16)
            att1_sb = CONST("att1_sb", att1r, (P, 2 * D1), dt.bfloat16)
            wlr2_sb = CONST("wlr2_sb", wlr2, (P, 32 * D2), dt.bfloat16)
            we2_sb  = CONST("we2_sb", we2, (F, D2), dt.bfloat16)
            att2_sb = CONST("att2_sb", att2r, (P, 2 * D2), dt.bfloat16)
            id_sb   = CONST("id_sb", ident, (P, P), dt.bfloat16)
            eaT_sb  = CONST("eaT_sb", eaT, (F, L), dt.bfloat16)
            s01_sb  = CONST("s01_sb", s01T, (P, L), dt.bfloat16)
            srci_sb = CONST("srci_sb", srci, (P, NCH), dt.int32)
            dstpi_sb = CONST("dstpi_sb", dstpi, (P, NCH), dt.int32)
            dstli_sb = CONST("dstli_sb", dstli, (P, NCH), dt.int32)
            ep2_sb = cp.tile([P, NCH * D2], dt.bfloat16, name="ep2_sb")

            # warm-up collective: absorbs CC cold-start under phase A/B
            nc.sync.dma_start(out=dum0[:], in_=xT[0:1, 0:8])
            nc.gpsimd.collective_compute(
                "AllGather", ALU.bypass, replica_groups=RG,
                ins=[dum0[:]], outs=[dumAG[:]])
            nc.sync.dma_start(out=dumsink[:], in_=dumAG[0:1])

            # ---------- phase A: node projections + ep2 precompute ----------
            with (
                tc.tile_pool(name="a_ps", bufs=3, space="PSUM") as aps,
                tc.tile_pool(name="a_ps2", bufs=1, space="PSUM") as aps2,
                tc.tile_pool(name="a_sb", bufs=4) as asb,
            ):
                NT = (N + P - 1) // P
                for t in range(NT + G):
                    if t < NT:  # xl1 for ALL nodes (replicated compute)
                        mt = min(P, N - t * P)
                        lhs = xT_sb[:, t * P:t * P + mt]
                        w = wl1_sb
                        dst_tab, r0 = xl1_tab, t * P
                    else:       # xr1 for own slice
                        g = t - NT
                        mt = GN
                        lhs = xsT_sb[:, g * P:g * P + mt]
                        w = wr1_sb
                        dst_tab, r0 = xr1_sl, g * P
                    for half in range(2):
                        ps = aps.tile([P, 1024], dt.float32, tag="ps")
                        for j in range(2):
                            nc.tensor.matmul(
                                out=ps[:mt, j * 512:(j + 1) * 512],
                                lhsT=lhs,
                                rhs=w[:, half * 1024 + j * 512:
                                      half * 1024 + (j + 1) * 512],
                                start=True, stop=True)
                        xsb = asb.tile([P, 1024], dt.bfloat16, tag="xsb")
                        if half == 0:
                            nc.scalar.copy(out=xsb[:mt], in_=ps[:mt])
                        else:
                            nc.vector.tensor_copy(out=xsb[:mt], in_=ps[:mt])
                        nc.sync.dma_start(
                            out=dst_tab[r0:r0 + mt,
                                        half * 1024:(half + 1) * 1024],
                            in_=xsb[:mt])
                # layer-2 edge projections for all chunks (resident)
                NB = (NCH + 15) // 16
                for blk in range(NB):
                    ps2 = aps2.tile([P, 512], dt.float32, tag="ep2")
                    n_in_blk = min(16, NCH - blk * 16)
                    for j in range(n_in_blk):
                        ch = blk * 16 + j
                        nc.tensor.matmul(
                            out=ps2[:, j * D2:(j + 1) * D2],
                            lhsT=eaT_sb[:, ch * P:(ch + 1) * P],
                            rhs=we2_sb[:], start=True, stop=True)
                    nc.vector.tensor_copy(
                        out=ep2_sb[:, blk * 512:blk * 512 + n_in_blk * D2],
                        in_=ps2[:, :n_in_blk * D2])

            # ---------- phase B: layer-1 edge pass ----------
            xr2_tiles = []
            q2_tiles = []
            with (tc.tile_pool(name="xr2res", bufs=G) as xr2p,
                  tc.tile_pool(name="c_q", bufs=G * ((CH + 1) // 2)) as cqp):
              with (
                tc.tile_pool(name="b_acc", bufs=1, space="PSUM") as accp,  # 4
                tc.tile_pool(name="b_ep", bufs=1, space="PSUM") as epp,    # 2
                tc.tile_pool(name="b_sm", bufs=1, space="PSUM") as smp,    # 1
                tc.tile_pool(name="b_x2", bufs=1, space="PSUM") as x2p,    # 1
                tc.tile_pool(name="b_q", bufs=4) as qp,
                tc.tile_pool(name="b_z", bufs=2) as zp,
                tc.tile_pool(name="b_m", bufs=2) as mp,
                tc.tile_pool(name="b_ma", bufs=1) as map_,
                tc.tile_pool(name="b_xls", bufs=2) as xlsp,
                tc.tile_pool(name="b_sc", bufs=4) as scp,
                tc.tile_pool(name="b_fin", bufs=1) as finp,
              ):
                qs = []
                for g in range(G):
                    acc = accp.tile([P, D1], dt.float32, tag="acc")
                    den = smp.tile([P, 8], dt.float32, tag="sm")
                    for pch in range((CH + 1) // 2):
                        chs = [c for c in (2 * pch, 2 * pch + 1) if c < CH]
                        np_ = len(chs)
                        mpair = mp.tile([P, np_ * D1], dt.bfloat16, tag="m")
                        for i, ch in enumerate(chs):
                            chb = g * CH + ch
                            e0 = chb * P
                            q = qp.tile([P, D1], dt.bfloat16, tag="q")
                            nc.gpsimd.indirect_dma_start(
                                out=q[:], out_offset=None, in_=xl1_tab[:],
                                in_offset=bass.IndirectOffsetOnAxis(
                                    ap=srci_sb[:, chb:chb + 1], axis=0))
                            nc.gpsimd.indirect_dma_start(
                                out=q[:], out_offset=None, in_=xr1_sl[:],
                                in_offset=bass.IndirectOffsetOnAxis(
                                    ap=dstpi_sb[:, chb:chb + 1], axis=0),
                                compute_op=ALU.add)
                            epb = zp.tile([P, D1], dt.bfloat16, tag="epb")
                            for half in range(2):
                                c0 = half * 1024
                                ep = epp.tile([P, 1024], dt.float32, tag="ep")
                                for j in range(2):
                                    nc.tensor.matmul(
                                        out=ep[:, j * 512:(j + 1) * 512],
                                        lhsT=eaT_sb[:, e0:e0 + P],
                                        rhs=we1_sb[:, c0 + j * 512:
                                                   c0 + (j + 1) * 512],
                                        start=True, stop=True)
                                nc.scalar.copy(out=epb[:, c0:c0 + 1024],
                                               in_=ep[:])
                            z = zp.tile([P, D1], dt.bfloat16, tag="z")
                            nc.vector.tensor_add(out=z[:], in0=q[:],
                                                 in1=epb[:])
                            nc.scalar.activation(
                                out=mpair[:, i * D1:(i + 1) * D1], in_=z[:],
                                func=AF.Prelu, alpha=NEG)
                            qs.append(q)
                        # fused logits for the pair: ma = m*att, blocked sum
                        nh = np_ * H1
                        ma = map_.tile([P, nh, 272], dt.bfloat16, tag="ma")
                        nc.vector.tensor_tensor(
                            out=ma[:, :, :C1],
                            in0=mpair[:].rearrange("p (g c) -> p g c", g=nh),
                            in1=att1_sb[:, :np_ * D1].rearrange(
                                "p (g c) -> p g c", g=nh),
                            op=ALU.mult)
                        logit = scp.tile([P, nh], dt.float32, tag="lg")
                        nc.vector.tensor_reduce(
                            out=logit[:], in_=ma[:, :, :C1],
                            axis=mybir.AxisListType.X, op=ALU.add)
                        ex = scp.tile([P, nh], dt.float32, tag="ex")
                        nc.scalar.activation(out=ex[:], in_=logit[:],
                                             func=AF.Exp)
                        exb = scp.tile([P, nh], dt.bfloat16, tag="exb")
                        nc.scalar.copy(out=exb[:], in_=ex[:])
                        for i, ch in enumerate(chs):
                            chb = g * CH + ch
                            e0 = chb * P
                            q = qs[chb]
                            xls = xlsp.tile([P, D1], dt.bfloat16, tag="xls")
                            for h in range(H1):
                                hh = i * H1 + h
                                if h % 4 < 2:
                                    nc.vector.tensor_scalar(
                                        out=xls[:, h * C1:(h + 1) * C1],
                                        in0=q[:, h * C1:(h + 1) * C1],
                                        scalar1=ex[:, hh:hh + 1],
                                        scalar2=None, op0=ALU.mult)
                                else:
                                    nc.scalar.activation(
                                        out=xls[:, h * C1:(h + 1) * C1],
                                        in_=q[:, h * C1:(h + 1) * C1],
                                        func=AF.Copy,
                                        scale=ex[:, hh:hh + 1])
                            for j in range(4):
                                nc.tensor.matmul(
                                    out=acc[:, j * 512:(j + 1) * 512],
                                    lhsT=s01_sb[:, e0:e0 + P],
                                    rhs=xls[:, j * 512:(j + 1) * 512],
                                    start=(ch == 0), stop=(ch == CH - 1))
                            nc.tensor.matmul(
                                out=den[:], lhsT=s01_sb[:, e0:e0 + P],
                                rhs=exb[:, i * H1:(i + 1) * H1],
                                start=(ch == 0), stop=(ch == CH - 1))

                    # ---- group finalize ----
                    dn = scp.tile([P, 8], dt.float32, tag="dn")
                    nc.vector.reciprocal(out=dn[:], in_=den[:])
                    xr_g = finp.tile([P, D1], dt.bfloat16, tag="xrg")
                    nc.sync.dma_start(out=xr_g[:],
                                      in_=xr1_sl[g * P:(g + 1) * P, :])
                    hs = finp.tile([P, D1], dt.bfloat16, tag="hs")
                    for h in range(H1):
                        nc.vector.scalar_tensor_tensor(
                            out=hs[:, h * C1:(h + 1) * C1],
                            in0=acc[:, h * C1:(h + 1) * C1],
                            scalar=dn[:, h:h + 1],
                            in1=xr_g[:, h * C1:(h + 1) * C1],
                            op0=ALU.mult, op1=ALU.subtract)
                    hr = finp.tile([P, D1], dt.bfloat16, tag="hr")
                    nc.scalar.activation(out=hr[:], in_=hs[:], func=AF.Relu)
                    x2ps = x2p.tile([P, 2 * D2], dt.float32, tag="x2")
                    for kk in range(16):
                        hT = finp.tile([P, P], dt.bfloat16, tag="hT", bufs=2)
                        nc.sync.dma_start(out=hT[:],
                                          in_=hr[:, kk * P:(kk + 1) * P],
                                          transpose=True)
                        nc.tensor.matmul(
                            out=x2ps[:, 0:2 * D2], lhsT=hT[:],
                            rhs=wlr2_sb[:, kk * 2 * D2:(kk + 1) * 2 * D2],
                            start=(kk == 0), stop=(kk == 15))
                    x2sb = finp.tile([P, 2 * D2], dt.float32, tag="x2sb")
                    nc.vector.tensor_copy(out=x2sb[:], in_=x2ps[:])
                    nc.sync.dma_start(out=xl2_own[g * GN:(g + 1) * GN, :],
                                      in_=x2sb[:GN, 0:D2])
                    nc.sync.dma_start(out=xr2_tabs[g][:],
                                      in_=x2sb[:, D2:2 * D2])
                    xr2_res = xr2p.tile([P, D2], dt.float32, tag="xr2")
                    nc.vector.tensor_copy(out=xr2_res[:], in_=x2sb[:, D2:])
                    xr2_tiles.append(xr2_res)
                    # phase-C xr2[dst] prefetch for this group's chunks:
                    # depends only on xr2_tabs[g], so it runs during phase B
                    for pch in range((CH + 1) // 2):
                        chs = [c for c in (2 * pch, 2 * pch + 1) if c < CH]
                        q2p = cqp.tile([P, 2, D2], dt.float32, tag="q2")
                        for i, ch in enumerate(chs):
                            chb = g * CH + ch
                            nc.gpsimd.indirect_dma_start(
                                out=q2p[:, i, :], out_offset=None,
                                in_=xr2_tabs[g][:],
                                in_offset=bass.IndirectOffsetOnAxis(
                                    ap=dstli_sb[:, chb:chb + 1], axis=0))
                        q2_tiles.append(q2p)

              # ---------- AllGather of xl2 ----------
              if True:
                nc.gpsimd.collective_compute(
                    "AllGather", ALU.bypass, replica_groups=RG,
                    ins=[xl2_own[:]], outs=[xl2_all[:]])

                # ---------- phase C: layer-2 edge pass ----------
                with (
                    tc.tile_pool(name="c_ps", bufs=2, space="PSUM") as cps,
                    tc.tile_pool(name="c_sb", bufs=4) as csb,
                ):
                    for g in range(G):
                        acc2 = cps.tile([P, D2 + 1], dt.float32, tag="a2")
                        for pch in range((CH + 1) // 2):
                            chs = [c for c in (2 * pch, 2 * pch + 1) if c < CH]
                            np_ = len(chs)
                            q2p = q2_tiles[g * ((CH + 1) // 2) + pch]
                            chb0 = g * CH + chs[0]
                            for i, ch in enumerate(chs):
                                chb = g * CH + ch
                                nc.gpsimd.indirect_dma_start(
                                    out=q2p[:, i, :], out_offset=None,
                                    in_=xl2_all[:],
                                    in_offset=bass.IndirectOffsetOnAxis(
                                        ap=srci_sb[:, chb:chb + 1], axis=0),
                                    compute_op=ALU.add)
                            z2 = csb.tile([P, np_ * D2], dt.float32, tag="z2")
                            nc.vector.tensor_add(
                                out=z2[:],
                                in0=q2p[:, :np_, :].rearrange(
                                    "p a b -> p (a b)"),
                                in1=ep2_sb[:, chb0 * D2:
                                           (chb0 + np_) * D2])
                            m2 = csb.tile([P, np_ * D2], dt.bfloat16,
                                          tag="m2")
                            nc.scalar.activation(out=m2[:], in_=z2[:],
                                                 func=AF.Prelu, alpha=NEG)
                            ma2 = csb.tile([P, np_, 40], dt.bfloat16,
                                           tag="ma2")
                            nc.vector.tensor_tensor(
                                out=ma2[:, :, :D2],
                                in0=m2[:].rearrange("p (a b) -> p a b",
                                                    a=np_),
                                in1=att2_sb[:, :np_ * D2].rearrange(
                                    "p (a b) -> p a b", a=np_),
                                op=ALU.mult)
                            lg2 = csb.tile([P, np_], dt.float32, tag="lg2")
                            nc.vector.tensor_reduce(
                                out=lg2[:], in_=ma2[:, :, :D2],
                                axis=mybir.AxisListType.X, op=ALU.add)
                            ex2 = csb.tile([P, np_], dt.float32, tag="ex2")
                            nc.scalar.activation(out=ex2[:], in_=lg2[:],
                                                 func=AF.Exp)
                            for i, ch in enumerate(chs):
                                chb = g * CH + ch
                                e0 = chb * P
                                xls2 = csb.tile([P, D2 + 1], dt.bfloat16,
                                                tag="xls2")
                                nc.scalar.copy(out=xls2[:, D2:D2 + 1],
                                               in_=ex2[:, i:i + 1])
                                nc.vector.tensor_scalar(
                                    out=xls2[:, :D2], in0=q2p[:, i, :],
                                    scalar1=ex2[:, i:i + 1], scalar2=None,
                                    op0=ALU.mult)
                                nc.tensor.matmul(
                                    out=acc2[:], lhsT=s01_sb[:, e0:e0 + P],
                                    rhs=xls2[:],
                                    start=(ch == 0), stop=(ch == CH - 1))
                        d2 = csb.tile([P, 1], dt.float32, tag="d2")
                        nc.vector.reciprocal(out=d2[:], in_=acc2[:, D2:D2 + 1])
                        o2 = csb.tile([P, D2], dt.float32, tag="o2")
                        nc.vector.scalar_tensor_tensor(
                            out=o2[:], in0=acc2[:, :D2], scalar=d2[:, :1],
                            in1=xr2_tiles[g], op0=ALU.mult, op1=ALU.subtract)
                        orl = csb.tile([P, D2], dt.float32, tag="orl")
                        nc.vector.tensor_scalar(
                            out=orl[:], in0=o2[:], scalar1=0.0, scalar2=None,
                            op0=ALU.max)
                        nc.sync.dma_start(out=out[g * GN:(g + 1) * GN, :],
                                          in_=orl[:GN])

    nc.compile()
    return nc


def _prep_inputs(x, edge_index, edge_attr, Wl1, bl1, Wr1, br1, We1, att1, b1,
                 Wl2, bl2, Wr2, br2, We2, att2, b2):
    for b in (bl1, br1, b1, bl2, br2, b2):
        assert not np.any(np.asarray(b)), "nonzero biases not implemented"

    src = np.asarray(edge_index[0], dtype=np.int64)
    dst = np.asarray(edge_index[1], dtype=np.int64)
    ea = np.asarray(edge_attr, dtype=np.float32)

    # PyG fill_value='mean' self loops, computed host-side
    cnt = np.bincount(dst, minlength=N).astype(np.float32)
    ssum = np.zeros((N, F), np.float32)
    np.add.at(ssum, dst, ea)
    self_attr = ssum / np.maximum(cnt, 1.0)[:, None]

    order = np.argsort(dst, kind="stable")
    s_src, s_dst, s_ea = src[order], dst[order], ea[order]
    bounds = np.searchsorted(s_dst, np.arange(0, N + GN, GN))
    cnts = np.diff(bounds)                       # real edges per group (80,)
    CH = int(np.max((cnts + GN + P - 1) // P))   # incl. GN self edges
    NCH = G * CH
    L = NCH * P

    x = np.asarray(x, dtype=np.float32)
    common = {
        "xT": x.T.astype(BF16),
        "wl1": np.asarray(Wl1, np.float32).astype(BF16),
        "wr1": np.asarray(Wr1, np.float32).astype(BF16),
        "we1": np.asarray(We1, np.float32).astype(BF16),
        "att1r": np.tile(np.asarray(att1, np.float32).reshape(1, D1),
                         (P, 2)).astype(BF16),
        "wlr2": np.concatenate([
                    np.asarray(Wl2, np.float32).reshape(16, P, D2),
                    np.asarray(Wr2, np.float32).reshape(16, P, D2)],
                    axis=2).transpose(1, 0, 2).reshape(P, 32 * D2)
                .astype(BF16),
        "we2": np.asarray(We2, np.float32).astype(BF16),
        "att2r": np.tile(np.asarray(att2, np.float32).reshape(1, D2),
                         (P, 2)).astype(BF16),
        "ident": np.eye(P, dtype=np.float32).astype(BF16),
    }

    in_maps = []
    for k in range(M):
        base_node = k * NPC
        ea_c = np.zeros((L, F), np.float32)
        s01_c = np.zeros((L, P), np.float32)
        srci_c = np.zeros((L,), np.int32)
        dstpi_c = np.zeros((L,), np.int32)
        for g in range(G):
            gb = base_node + g * GN
            lo, hi = bounds[k * G + g], bounds[k * G + g + 1]
            cnt_g = hi - lo
            tot = cnt_g + GN
            assert tot <= CH * P
            o0 = g * CH * P
            sl = np.arange(o0, o0 + tot)
            ea_c[sl[:cnt_g]] = s_ea[lo:hi]
            ea_c[sl[cnt_g:]] = self_attr[gb:gb + GN]
            dl = np.concatenate([(s_dst[lo:hi] - gb), np.arange(GN)])
            s01_c[sl, dl] = 1.0
            srci_c[sl] = np.concatenate([s_src[lo:hi], np.arange(gb, gb + GN)])
            dstpi_c[sl] = g * P + dl
        im = dict(common)
        im["xsT"] = np.ascontiguousarray(
            np.pad(x[base_node:base_node + NPC].T.reshape(F, G, GN),
                   ((0, 0), (0, 0), (0, P - GN))).reshape(F, G * P)).astype(BF16)
        im["eaT"] = np.ascontiguousarray(ea_c.T).astype(BF16)
        # [p, ch*128+d] layout: edge slot p of chunk ch
        im["s01T"] = np.ascontiguousarray(
            s01_c.reshape(NCH, P, P).transpose(1, 0, 2)
            .reshape(P, L)).astype(BF16)
        im["srci"] = np.ascontiguousarray(srci_c.reshape(NCH, P).T)
        im["dstpi"] = np.ascontiguousarray(dstpi_c.reshape(NCH, P).T)
        im["dstli"] = np.ascontiguousarray((dstpi_c % P).reshape(NCH, P).T)
        in_maps.append(im)
    return in_maps, CH


_PROG_CACHE = {}


def _get_program(CH):
    if CH not in _PROG_CACHE:
        _PROG_CACHE[CH] = _build_program(CH)
    return _PROG_CACHE[CH]


def run(inputs, trace=False, tmpdir=None):
    in_maps, CH = _prep_inputs(**inputs)
    nc = _get_program(CH)
    res = run_bass_kernel_spmd(nc, in_maps, list(range(M)), trace=trace,
                               tmpdir=tmpdir)
    outp = np.concatenate([res.results[k]["out"] for k in range(M)], axis=0)
    return outp.astype(np.float32), res


def kernel(**inputs):
    outp, _ = run(inputs)
    return outp


# revision 27
# speedup vs baseline: 1.0361x; 1.0361x over previous
"""GATv2 2-layer GNN on 8 Trainium2 NeuronCores (Bass/Tile, edge-parallel).

Sharding: edges sorted by dst node, dst-range sharded across 8 cores
(core k owns dst nodes [1250k, 1250(k+1))), so the per-dst segment
softmax and aggregation are fully core-local. Self-loop edge attrs
(mean of incoming) are precomputed host-side and folded into the edge
stream. Layer-1 node projections are computed replicated (xl1 for all
nodes; xr1 for the own slice). Between layers only the 32-wide xl2
projection is exchanged with a single AllGather (xr2 is dst-local).

Per-edge-chunk pipeline (128 edges):
  gpsimd: q = xl1[src] (+)DMA xr1[dst]          (fused add-gather, bf16)
  tensor: ep = ea @ We1 into PSUM (2x 1024-wide matmuls)
  vector: z = q + ep ; 8x logit-accum STT ; 8x xls = q*ex
  scalar: m = LeakyRelu(z) ; ex = Exp(logits)
  tensor: acc += s01^T @ xls ; den += s01^T @ ex
Aggregation identity: sum_e alpha*(xl+xr) = out[d] + xr[d] (softmax
sums to 1), so the group finalize computes h = relu(acc/den - xr[d])
and xl never needs to be gathered standalone.
"""
import sys
sys.path.insert(0, "/opt/trn_rl_repo")

import numpy as np
import ml_dtypes

import concourse.bass as bass
import concourse.bacc as bacc
import concourse.tile as tile
from concourse import mybir
from concourse.bass_utils import run_bass_kernel_spmd

BF16 = ml_dtypes.bfloat16

N, E, F = 10000, 80000, 128
H1, C1 = 8, 256
D1 = H1 * C1          # 2048
D2 = 32               # layer-2 out (1 head)
NEG = 0.2
M = 8                 # cores
NPC = N // M          # 1250 nodes per core
GN = 125              # dst nodes per group
G = NPC // GN         # 10 groups per core
P = 128

dt = mybir.dt


def _build_program(CH):
    """Build the SPMD Bass program. CH = chunks per group (self edges incl)."""
    NCH = G * CH          # chunks per core
    L = NCH * P           # edge-stream length per core
    nc = bacc.Bacc("TRN2", target_bir_lowering=False, debug=False, num_devices=M)

    ei = {}
    def EIN(name, shape, dtype):
        ei[name] = nc.dram_tensor(name, list(shape), dtype, kind="ExternalInput")
        return ei[name]

    xT    = EIN("xT",    (P, N),       dt.bfloat16)   # x transposed (replicated)
    xsT   = EIN("xsT",   (P, G * P),   dt.bfloat16)   # own-slice cols, group padded
    wl1   = EIN("wl1",   (F, D1),      dt.bfloat16)
    wr1   = EIN("wr1",   (F, D1),      dt.bfloat16)
    we1   = EIN("we1",   (F, D1),      dt.bfloat16)
    att1r = EIN("att1r", (P, 2 * D1),  dt.bfloat16)   # att1 rows, tiled twice
    wlr2  = EIN("wlr2",  (P, 16 * 2 * D2), dt.bfloat16)  # [p, k*64+c]: Wl2|Wr2
    we2   = EIN("we2",   (F, D2),      dt.bfloat16)
    att2r = EIN("att2r", (P, 2 * D2),  dt.bfloat16)   # att2 rows, tiled twice
    ident = EIN("ident", (P, P),       dt.bfloat16)   # identity (PE transpose)
    eaT   = EIN("eaT",   (F, L),       dt.bfloat16)   # edge_attr^T, sorted+padded
    s01T  = EIN("s01T",  (P, L),       dt.bfloat16)   # [p, ch*128+d]: edge p of
                                                      # chunk ch one-hot on dst d
    srci  = EIN("srci",  (P, NCH),     dt.int32)      # global src id per slot
    dstpi = EIN("dstpi", (P, NCH),     dt.int32)      # group-padded local dst id
    dstli = EIN("dstli", (P, NCH),     dt.int32)      # within-group local dst id

    out = nc.dram_tensor("out", [NPC, D2], dt.float32, kind="ExternalOutput")

    # DRAM scratch
    xl1_tab = nc.dram_tensor("xl1_tab", [N, D1], dt.bfloat16)
    xr1_sl  = nc.dram_tensor("xr1_sl", [G * P, D1], dt.bfloat16)
    xl2_own = nc.dram_tensor("xl2_own", [NPC, D2], dt.float32)
    xr2_tabs = [nc.dram_tensor(f"xr2_tab{g}", [P, D2], dt.float32)
                for g in range(G)]
    xl2_all = nc.dram_tensor("xl2_all", [N, D2], dt.float32, addr_space="Shared")
    dum0    = nc.dram_tensor("dum0", [1, 8], dt.bfloat16)
    dumAG   = nc.dram_tensor("dumAG", [M, 8], dt.bfloat16, addr_space="Shared")
    dumsink = nc.dram_tensor("dumsink", [1, 8], dt.bfloat16)

    AF = mybir.ActivationFunctionType
    ALU = mybir.AluOpType
    RG = [list(range(M))]

    with tile.TileContext(nc) as tc:
        with tc.tile_pool(name="consts", bufs=1) as cp:
            def CONST(name, src, shape, dtype):
                t = cp.tile(list(shape), dtype, name=name)
                nc.sync.dma_start(out=t[:], in_=src[:])
                return t

            we1_sb  = CONST("we1_sb", we1, (F, D1), dt.bfloat16)
            att1_sb = CONST("att1_sb", att1r, (P, 2 * D1), dt.bfloat16)
            wlr2_sb = CONST("wlr2_sb", wlr2, (P, 32 * D2), dt.bfloat16)
            we2_sb  = CONST("we2_sb", we2, (F, D2), dt.bfloat16)
            att2_sb = CONST("att2_sb", att2r, (P, 2 * D2), dt.bfloat16)
            id_sb   = CONST("id_sb", ident, (P, P), dt.bfloat16)
            eaT_sb  = CONST("eaT_sb", eaT, (F, L), dt.bfloat16)
            s01_sb  = CONST("s01_sb", s01T, (P, L), dt.bfloat16)
            srci_sb = CONST("srci_sb", srci, (P, NCH), dt.int32)
            dstpi_sb = CONST("dstpi_sb", dstpi, (P, NCH), dt.int32)
            dstli_sb = CONST("dstli_sb", dstli, (P, NCH), dt.int32)
            ep2_sb = cp.tile([P, NCH * D2], dt.bfloat16, name="ep2_sb")

            # warm-up collective: absorbs CC cold-start under phase A/B
            nc.sync.dma_start(out=dum0[:], in_=xT[0:1, 0:8])
            nc.gpsimd.collective_compute(
                "AllGather", ALU.bypass, replica_groups=RG,
                ins=[dum0[:]], outs=[dumAG[:]])
            nc.sync.dma_start(out=dumsink[:], in_=dumAG[0:1])

            # ---------- phase A: node projections + ep2 precompute ----------
            with (
                tc.tile_pool(name="a_consts", bufs=1) as acp,
                tc.tile_pool(name="a_ps", bufs=3, space="PSUM") as aps,
                tc.tile_pool(name="a_ps2", bufs=1, space="PSUM") as aps2,
                tc.tile_pool(name="a_sb", bufs=4) as asb,
            ):
                xT_sb = acp.tile([P, N], dt.bfloat16, name="xT_sb")
                nc.sync.dma_start(out=xT_sb[:], in_=xT[:])
                xsT_sb = acp.tile([P, G * P], dt.bfloat16, name="xsT_sb")
                nc.sync.dma_start(out=xsT_sb[:], in_=xsT[:])
                wl1_sb = acp.tile([F, D1], dt.bfloat16, name="wl1_sb")
                nc.sync.dma_start(out=wl1_sb[:], in_=wl1[:])
                wr1_sb = acp.tile([F, D1], dt.bfloat16, name="wr1_sb")
                nc.sync.dma_start(out=wr1_sb[:], in_=wr1[:])
                NT = (N + P - 1) // P
                for t in range(NT + G):
                    if t < NT:  # xl1 for ALL nodes (replicated compute)
                        mt = min(P, N - t * P)
                        lhs = xT_sb[:, t * P:t * P + mt]
                        w = wl1_sb
                        dst_tab, r0 = xl1_tab, t * P
                    else:       # xr1 for own slice
                        g = t - NT
                        mt = GN
                        lhs = xsT_sb[:, g * P:g * P + mt]
                        w = wr1_sb
                        dst_tab, r0 = xr1_sl, g * P
                    for half in range(2):
                        ps = aps.tile([P, 1024], dt.float32, tag="ps")
                        for j in range(2):
                            nc.tensor.matmul(
                                out=ps[:mt, j * 512:(j + 1) * 512],
                                lhsT=lhs,
                                rhs=w[:, half * 1024 + j * 512:
                                      half * 1024 + (j + 1) * 512],
                                start=True, stop=True)
                        xsb = asb.tile([P, 1024], dt.bfloat16, tag="xsb")
                        if half == 0:
                            nc.scalar.copy(out=xsb[:mt], in_=ps[:mt])
                        else:
                            nc.vector.tensor_copy(out=xsb[:mt], in_=ps[:mt])
                        nc.sync.dma_start(
                            out=dst_tab[r0:r0 + mt,
                                        half * 1024:(half + 1) * 1024],
                            in_=xsb[:mt])
                # layer-2 edge projections for all chunks (resident)
                NB = (NCH + 15) // 16
                for blk in range(NB):
                    ps2 = aps2.tile([P, 512], dt.float32, tag="ep2")
                    n_in_blk = min(16, NCH - blk * 16)
                    for j in range(n_in_blk):
                        ch = blk * 16 + j
                        nc.tensor.matmul(
                            out=ps2[:, j * D2:(j + 1) * D2],
                            lhsT=eaT_sb[:, ch * P:(ch + 1) * P],
                            rhs=we2_sb[:], start=True, stop=True)
                    nc.vector.tensor_copy(
                        out=ep2_sb[:, blk * 512:blk * 512 + n_in_blk * D2],
                        in_=ps2[:, :n_in_blk * D2])

            # ---------- phase B: layer-1 edge pass ----------
            xr2_tiles = []
            q2_tiles = []
            with (tc.tile_pool(name="xr2res", bufs=G) as xr2p,
                  tc.tile_pool(name="c_q", bufs=G * ((CH + 1) // 2)) as cqp):
              with (
                tc.tile_pool(name="b_acc", bufs=1, space="PSUM") as accp,  # 4
                tc.tile_pool(name="b_ep", bufs=1, space="PSUM") as epp,    # 2
                tc.tile_pool(name="b_sm", bufs=1, space="PSUM") as smp,    # 1
                tc.tile_pool(name="b_q", bufs=6) as qp,
                tc.tile_pool(name="b_z", bufs=3) as zp,
                tc.tile_pool(name="b_m", bufs=3) as mp,
                tc.tile_pool(name="b_ma", bufs=2) as map_,
                tc.tile_pool(name="b_xls", bufs=3) as xlsp,
                tc.tile_pool(name="b_sc", bufs=4) as scp,
                tc.tile_pool(name="b_fin", bufs=2) as finp,
              ):
                qs = []
                for g in range(G):
                    acc = accp.tile([P, D1], dt.float32, tag="acc")
                    sx = smp.tile([P, 72], dt.float32, tag="sm")
                    den = sx[:, 0:8]
                    for pch in range((CH + 1) // 2):
                        chs = [c for c in (2 * pch, 2 * pch + 1) if c < CH]
                        np_ = len(chs)
                        mpair = mp.tile([P, np_ * D1], dt.bfloat16, tag="m")
                        for i, ch in enumerate(chs):
                            chb = g * CH + ch
                            e0 = chb * P
                            q = qp.tile([P, D1], dt.bfloat16, tag="q")
                            nc.gpsimd.indirect_dma_start(
                                out=q[:], out_offset=None, in_=xl1_tab[:],
                                in_offset=bass.IndirectOffsetOnAxis(
                                    ap=srci_sb[:, chb:chb + 1], axis=0))
                            nc.gpsimd.indirect_dma_start(
                                out=q[:], out_offset=None, in_=xr1_sl[:],
                                in_offset=bass.IndirectOffsetOnAxis(
                                    ap=dstpi_sb[:, chb:chb + 1], axis=0),
                                compute_op=ALU.add)
                            z = zp.tile([P, D1], dt.bfloat16, tag="z")
                            for half in range(2):
                                c0 = half * 1024
                                ep = epp.tile([P, 1024], dt.float32, tag="ep")
                                for j in range(2):
                                    nc.tensor.matmul(
                                        out=ep[:, j * 512:(j + 1) * 512],
                                        lhsT=eaT_sb[:, e0:e0 + P],
                                        rhs=we1_sb[:, c0 + j * 512:
                                                   c0 + (j + 1) * 512],
                                        start=True, stop=True)
                                nc.vector.tensor_add(
                                    out=z[:, c0:c0 + 1024],
                                    in0=q[:, c0:c0 + 1024], in1=ep[:])
                                nc.scalar.activation(
                                    out=mpair[:, i * D1 + c0:
                                              i * D1 + c0 + 1024],
                                    in_=z[:, c0:c0 + 1024],
                                    func=AF.Prelu, alpha=NEG)
                            qs.append(q)
                        # fused logits for the pair: ma = m*att, blocked sum
                        nh = np_ * H1
                        ma = map_.tile([P, nh, 272], dt.bfloat16, tag="ma")
                        nc.vector.tensor_tensor(
                            out=ma[:, :, :C1],
                            in0=mpair[:].rearrange("p (g c) -> p g c", g=nh),
                            in1=att1_sb[:, :np_ * D1].rearrange(
                                "p (g c) -> p g c", g=nh),
                            op=ALU.mult)
                        logit = scp.tile([P, nh], dt.bfloat16, tag="lg")
                        with nc.allow_low_precision("fp32 internal accum; "
                                                    "bf16 rounds output only"):
                            nc.vector.tensor_reduce(
                                out=logit[:], in_=ma[:, :, :C1],
                                axis=mybir.AxisListType.X, op=ALU.add)
                        ex = scp.tile([P, nh], dt.float32, tag="ex")
                        nc.scalar.activation(out=ex[:], in_=logit[:],
                                             func=AF.Exp)
                        exb = scp.tile([P, nh], dt.bfloat16, tag="exb")
                        nc.scalar.copy(out=exb[:], in_=ex[:])
                        for i, ch in enumerate(chs):
                            chb = g * CH + ch
                            e0 = chb * P
                            q = qs[chb]
                            xls = xlsp.tile([P, D1], dt.bfloat16, tag="xls")
                            for h in range(H1):
                                hh = i * H1 + h
                                if h % 4 < 2:
                                    nc.vector.tensor_scalar(
                                        out=xls[:, h * C1:(h + 1) * C1],
                                        in0=q[:, h * C1:(h + 1) * C1],
                                        scalar1=ex[:, hh:hh + 1],
                                        scalar2=None, op0=ALU.mult)
                                else:
                                    nc.scalar.activation(
                                        out=xls[:, h * C1:(h + 1) * C1],
                                        in_=q[:, h * C1:(h + 1) * C1],
                                        func=AF.Copy,
                                        scale=ex[:, hh:hh + 1])
                            for j in range(4):
                                nc.tensor.matmul(
                                    out=acc[:, j * 512:(j + 1) * 512],
                                    lhsT=s01_sb[:, e0:e0 + P],
                                    rhs=xls[:, j * 512:(j + 1) * 512],
                                    start=(ch == 0), stop=(ch == CH - 1))
                            nc.tensor.matmul(
                                out=den[:], lhsT=s01_sb[:, e0:e0 + P],
                                rhs=exb[:, i * H1:(i + 1) * H1],
                                start=(ch == 0), stop=(ch == CH - 1))

                    # ---- group finalize ----
                    dn = scp.tile([P, 8], dt.float32, tag="dn")
                    nc.vector.reciprocal(out=dn[:], in_=den[:])
                    xr_g = finp.tile([P, D1], dt.bfloat16, tag="xrg")
                    nc.sync.dma_start(out=xr_g[:],
                                      in_=xr1_sl[g * P:(g + 1) * P, :])
                    hs = finp.tile([P, D1], dt.bfloat16, tag="hs")
                    for h in range(H1):
                        nc.vector.scalar_tensor_tensor(
                            out=hs[:, h * C1:(h + 1) * C1],
                            in0=acc[:, h * C1:(h + 1) * C1],
                            scalar=dn[:, h:h + 1],
                            in1=xr_g[:, h * C1:(h + 1) * C1],
                            op0=ALU.mult, op1=ALU.subtract)
                    hr = finp.tile([P, D1], dt.bfloat16, tag="hr")
                    nc.scalar.activation(out=hr[:], in_=hs[:], func=AF.Relu)
                    x2ps = sx[:, 8:8 + 2 * D2]
                    for kk in range(16):
                        hT = finp.tile([P, P], dt.bfloat16, tag="hT", bufs=2)
                        nc.sync.dma_start(out=hT[:],
                                          in_=hr[:, kk * P:(kk + 1) * P],
                                          transpose=True)
                        nc.tensor.matmul(
                            out=x2ps[:, 0:2 * D2], lhsT=hT[:],
                            rhs=wlr2_sb[:, kk * 2 * D2:(kk + 1) * 2 * D2],
                            start=(kk == 0), stop=(kk == 15))
                    x2sb = finp.tile([P, 2 * D2], dt.float32, tag="x2sb")
                    nc.vector.tensor_copy(out=x2sb[:], in_=x2ps[:])
                    nc.sync.dma_start(out=xl2_own[g * GN:(g + 1) * GN, :],
                                      in_=x2sb[:GN, 0:D2])
                    nc.sync.dma_start(out=xr2_tabs[g][:],
                                      in_=x2sb[:, D2:2 * D2])
                    xr2_res = xr2p.tile([P, D2], dt.float32, tag="xr2")
                    nc.vector.tensor_copy(out=xr2_res[:], in_=x2sb[:, D2:])
                    xr2_tiles.append(xr2_res)
                    # phase-C xr2[dst] prefetch for this group's chunks:
                    # depends only on xr2_tabs[g], so it runs during phase B
                    for pch in range((CH + 1) // 2):
                        chs = [c for c in (2 * pch, 2 * pch + 1) if c < CH]
                        q2p = cqp.tile([P, 2, D2], dt.float32, tag="q2")
                        for i, ch in enumerate(chs):
                            chb = g * CH + ch
                            nc.gpsimd.indirect_dma_start(
                                out=q2p[:, i, :], out_offset=None,
                                in_=xr2_tabs[g][:],
                                in_offset=bass.IndirectOffsetOnAxis(
                                    ap=dstli_sb[:, chb:chb + 1], axis=0))
                        q2_tiles.append(q2p)

              # ---------- AllGather of xl2 ----------
              if True:
                nc.gpsimd.collective_compute(
                    "AllGather", ALU.bypass, replica_groups=RG,
                    ins=[xl2_own[:]], outs=[xl2_all[:]])

                # ---------- phase C: layer-2 edge pass ----------
                with (
                    tc.tile_pool(name="c_ps", bufs=2, space="PSUM") as cps,
                    tc.tile_pool(name="c_sb", bufs=4) as csb,
                ):
                    for g in range(G):
                        acc2 = cps.tile([P, D2 + 1], dt.float32, tag="a2")
                        for pch in range((CH + 1) // 2):
                            chs = [c for c in (2 * pch, 2 * pch + 1) if c < CH]
                            np_ = len(chs)
                            q2p = q2_tiles[g * ((CH + 1) // 2) + pch]
                            chb0 = g * CH + chs[0]
                            for i, ch in enumerate(chs):
                                chb = g * CH + ch
                                nc.gpsimd.indirect_dma_start(
                                    out=q2p[:, i, :], out_offset=None,
                                    in_=xl2_all[:],
                                    in_offset=bass.IndirectOffsetOnAxis(
                                        ap=srci_sb[:, chb:chb + 1], axis=0),
                                    compute_op=ALU.add)
                            z2 = csb.tile([P, np_ * D2], dt.float32, tag="z2")
                            nc.vector.tensor_add(
                                out=z2[:],
                                in0=q2p[:, :np_, :].rearrange(
                                    "p a b -> p (a b)"),
                                in1=ep2_sb[:, chb0 * D2:
                                           (chb0 + np_) * D2])
                            m2 = csb.tile([P, np_ * D2], dt.bfloat16,
                                          tag="m2")
                            nc.scalar.activation(out=m2[:], in_=z2[:],
                                                 func=AF.Prelu, alpha=NEG)
                            ma2 = csb.tile([P, np_, 40], dt.bfloat16,
                                           tag="ma2")
                            nc.vector.tensor_tensor(
                                out=ma2[:, :, :D2],
                                in0=m2[:].rearrange("p (a b) -> p a b",
                                                    a=np_),
                                in1=att2_sb[:, :np_ * D2].rearrange(
                                    "p (a b) -> p a b", a=np_),
                                op=ALU.mult)
                            lg2 = csb.tile([P, np_], dt.float32, tag="lg2")
                            nc.vector.tensor_reduce(
                                out=lg2[:], in_=ma2[:, :, :D2],
                                axis=mybir.AxisListType.X, op=ALU.add)
                            ex2 = csb.tile([P, np_], dt.float32, tag="ex2")
                            nc.scalar.activation(out=ex2[:], in_=lg2[:],
                                                 func=AF.Exp)
                            for i, ch in enumerate(chs):
                                chb = g * CH + ch
                                e0 = chb * P
                                xls2 = csb.tile([P, D2 + 1], dt.bfloat16,
                                                tag="xls2")
                                nc.scalar.copy(out=xls2[:, D2:D2 + 1],
                                               in_=ex2[:, i:i + 1])
                                nc.vector.tensor_scalar(
                                    out=xls2[:, :D2], in0=q2p[:, i, :],
                                    scalar1=ex2[:, i:i + 1], scalar2=None,
                                    op0=ALU.mult)
                                nc.tensor.matmul(
                                    out=acc2[:], lhsT=s01_sb[:, e0:e0 + P],
                                    rhs=xls2[:],
                                    start=(ch == 0), stop=(ch == CH - 1))
                        d2 = csb.tile([P, 1], dt.float32, tag="d2")
                        nc.vector.reciprocal(out=d2[:], in_=acc2[:, D2:D2 + 1])
                        o2 = csb.tile([P, D2], dt.float32, tag="o2")
                        nc.vector.scalar_tensor_tensor(
                            out=o2[:], in0=acc2[:, :D2], scalar=d2[:, :1],
                            in1=xr2_tiles[g], op0=ALU.mult, op1=ALU.subtract)
                        orl = csb.tile([P, D2], dt.float32, tag="orl")
                        nc.vector.tensor_scalar(
                            out=orl[:], in0=o2[:], scalar1=0.0, scalar2=None,
                            op0=ALU.max)
                        nc.sync.dma_start(out=out[g * GN:(g + 1) * GN, :],
                                          in_=orl[:GN])

    nc.compile()
    return nc


def _prep_inputs(x, edge_index, edge_attr, Wl1, bl1, Wr1, br1, We1, att1, b1,
                 Wl2, bl2, Wr2, br2, We2, att2, b2):
    for b in (bl1, br1, b1, bl2, br2, b2):
        assert not np.any(np.asarray(b)), "nonzero biases not implemented"

    src = np.asarray(edge_index[0], dtype=np.int64)
    dst = np.asarray(edge_index[1], dtype=np.int64)
    ea = np.asarray(edge_attr, dtype=np.float32)

    # PyG fill_value='mean' self loops, computed host-side
    cnt = np.bincount(dst, minlength=N).astype(np.float32)
    ssum = np.zeros((N, F), np.float32)
    np.add.at(ssum, dst, ea)
    self_attr = ssum / np.maximum(cnt, 1.0)[:, None]

    order = np.argsort(dst, kind="stable")
    s_src, s_dst, s_ea = src[order], dst[order], ea[order]
    bounds = np.searchsorted(s_dst, np.arange(0, N + GN, GN))
    cnts = np.diff(bounds)                       # real edges per group (80,)
    CH = int(np.max((cnts + GN + P - 1) // P))   # incl. GN self edges
    NCH = G * CH
    L = NCH * P

    x = np.asarray(x, dtype=np.float32)
    common = {
        "xT": x.T.astype(BF16),
        "wl1": np.asarray(Wl1, np.float32).astype(BF16),
        "wr1": np.asarray(Wr1, np.float32).astype(BF16),
        "we1": np.asarray(We1, np.float32).astype(BF16),
        "att1r": np.tile(np.asarray(att1, np.float32).reshape(1, D1),
                         (P, 2)).astype(BF16),
        "wlr2": np.concatenate([
                    np.asarray(Wl2, np.float32).reshape(16, P, D2),
                    np.asarray(Wr2, np.float32).reshape(16, P, D2)],
                    axis=2).transpose(1, 0, 2).reshape(P, 32 * D2)
                .astype(BF16),
        "we2": np.asarray(We2, np.float32).astype(BF16),
        "att2r": np.tile(np.asarray(att2, np.float32).reshape(1, D2),
                         (P, 2)).astype(BF16),
        "ident": np.eye(P, dtype=np.float32).astype(BF16),
    }

    in_maps = []
    for k in range(M):
        base_node = k * NPC
        ea_c = np.zeros((L, F), np.float32)
        s01_c = np.zeros((L, P), np.float32)
        srci_c = np.zeros((L,), np.int32)
        dstpi_c = np.zeros((L,), np.int32)
        for g in range(G):
            gb = base_node + g * GN
            lo, hi = bounds[k * G + g], bounds[k * G + g + 1]
            cnt_g = hi - lo
            tot = cnt_g + GN
            assert tot <= CH * P
            o0 = g * CH * P
            sl = np.arange(o0, o0 + tot)
            ea_c[sl[:cnt_g]] = s_ea[lo:hi]
            ea_c[sl[cnt_g:]] = self_attr[gb:gb + GN]
            dl = np.concatenate([(s_dst[lo:hi] - gb), np.arange(GN)])
            s01_c[sl, dl] = 1.0
            srci_c[sl] = np.concatenate([s_src[lo:hi], np.arange(gb, gb + GN)])
            dstpi_c[sl] = g * P + dl
        im = dict(common)
        im["xsT"] = np.ascontiguousarray(
            np.pad(x[base_node:base_node + NPC].T.reshape(F, G, GN),
                   ((0, 0), (0, 0), (0, P - GN))).reshape(F, G * P)).astype(BF16)
        im["eaT"] = np.ascontiguousarray(ea_c.T).astype(BF16)
        # [p, ch*128+d] layout: edge slot p of chunk ch
        im["s01T"] = np.ascontiguousarray(
            s01_c.reshape(NCH, P, P).transpose(1, 0, 2)
            .reshape(P, L)).astype(BF16)
        im["srci"] = np.ascontiguousarray(srci_c.reshape(NCH, P).T)
        im["dstpi"] = np.ascontiguousarray(dstpi_c.reshape(NCH, P).T)
        im["dstli"] = np.ascontiguousarray((dstpi_c % P).reshape(NCH, P).T)
        in_maps.append(im)
    return in_maps, CH


_PROG_CACHE = {}


def _get_program(CH):
    if CH not in _PROG_CACHE:
        _PROG_CACHE[CH] = _build_program(CH)
    return _PROG_CACHE[CH]


def run(inputs, trace=False, tmpdir=None):
    in_maps, CH = _prep_inputs(**inputs)
    nc = _get_program(CH)
    res = run_bass_kernel_spmd(nc, in_maps, list(range(M)), trace=trace,
                               tmpdir=tmpdir)
    outp = np.concatenate([res.results[k]["out"] for k in range(M)], axis=0)
    return outp.astype(np.float32), res


def kernel(**inputs):
    outp, _ = run(inputs)
    return outp
